# revision 29
# baseline (speedup 1.0000x reference)
"""BoneCloud RBF-skinning kernel for 8 trn2 NeuronCores.

pred[n] = (sum_k u[n,k] * T_k @ [x_n,1]) / (sum_k u[n,k]),  u = exp(-sigma*dist(x_n, b_k))

Data-parallel over points: each of the 8 cores processes N/8 points; bone data
is replicated. Per core, per 512-point tile:
  1. PE: 4 K=16 bf16 matmuls compute p = -d2/2 for all 512 bones.
     Split-precision: x, b, |x|^2, |b|^2 are (hi,lo) bf16 pairs and the
     contraction carries all four cross terms, so p is fp32-accurate while
     the moving operand streams at bf16 rate. -> PSUM [128bones x 4*256pts]
  2. ACT: s = Sqrt(-2*p + eps) -> SBUF bf16 (bones x points layout)
  3. DVE: per-group max(s, 0) — DVE max is NaN-non-propagating, so sqrt(neg)
     from fp cancellation at coincident point/bone pairs becomes s=0 exactly
  4. ACT: Exp(-sigma*s) per 8-tile group, in place (ACT stream is order-pinned
     so the sqrt<->exp table-set switch happens once per 33-tile chunk)
  5. PE: blend matmul u^T @ [T_bf16 + T_resid | 1] with main+resid pairs
     accumulated into the same PSUM block, 16 subtiles per PSUM bank
     (col 16 = softmax normalizer Z)
  6. DVE: per-point 4x4 apply + divide by Z, batched over 8 tiles, reading
     R/T/Z straight from PSUM -> out
Blend/apply work is queued as micro-tasks and drained into the PE's idle time
between dist matmuls (the dist->sqrt->dist chain is ACT-paced), so PE and ACT
overlap throughout. DMA instruction counts are minimized (per-instruction
sequencer issue overhead is the limiter) and split between the sync (xyzq)
and gpsimd (xyz gather / out scatter) queues.
"""

from collections import deque

import numpy as np

import concourse.bacc as bacc
import concourse.mybir as mybir
import concourse.tile as tile
from concourse.bass_utils import run_bass_kernel_spmd
from concourse.tile_rust import add_dep_helper

SIGMA = 20.0
EPS = 1e-6
N_CORES = 8
PTS_TILE = 256
NB = 512  # bones
KD = 16  # dist contraction rows
G_MAX = 33  # point-tiles per ACT chunk
GRP = 8  # point-tiles per group (xq DMA / blend / apply batching)
TASKS_PER_SLOT = 3

_NC_CACHE = {}


def _chunks(n_tiles, g_max):
    out = []
    while n_tiles > 0:
        g = min(g_max, n_tiles)
        out.append(g)
        n_tiles -= g
    return out


def build_nc(npc, g_max=G_MAX, num_devices=N_CORES):
    """Build + compile the per-core SPMD program for npc points (npc % 512 == 0)."""
    key = (npc, g_max, num_devices)
    if key in _NC_CACHE:
        return _NC_CACHE[key]
    assert npc % PTS_TILE == 0
    n_tiles = npc // PTS_TILE
    chunks = _chunks(n_tiles, g_max)
    dt = mybir.dt
    af = mybir.ActivationFunctionType

    nc = bacc.Bacc("TRN2", target_bir_lowering=False, debug=False,
                   num_devices=num_devices)
    xyzq = nc.dram_tensor("xyzq13", [KD, npc], dt.bfloat16, kind="ExternalInput").ap()
    xyz3 = nc.dram_tensor("xyz3", [npc, 3], dt.float32, kind="ExternalInput").ap()
    bq = nc.dram_tensor("bonesq", [KD, 512], dt.bfloat16,
                        kind="ExternalInput").ap()
    tf = nc.dram_tensor("transf34", [128, 136], dt.bfloat16, kind="ExternalInput").ap()
    out3 = nc.dram_tensor("out3", [npc, 3], dt.float32, kind="ExternalOutput").ap()

    with tile.TileContext(nc) as tc:
        with (
            tc.tile_pool(name="const", bufs=1) as constp,
            tc.tile_pool(name="xq", bufs=3) as xqp,
            tc.tile_pool(name="ubuf", bufs=2) as ubp,
            tc.tile_pool(name="appl", bufs=3) as app,
            tc.tile_pool(name="psd", bufs=3, space="PSUM") as psdp,
            tc.tile_pool(name="psb", bufs=2, space="PSUM") as psbp,
        ):
            eps_sb = constp.tile([128, 1], dt.float32, tag="eps")
            nc.vector.memset(eps_sb[:], EPS)
            bq_sb = constp.tile([128, 512], dt.bfloat16, tag="bq")
            nc.sync.dma_start(out=bq_sb[0:KD, :], in_=bq[:, :])
            tf_sb = constp.tile([128, 136], dt.bfloat16, tag="tf")
            nc.sync.dma_start(out=tf_sb[:], in_=tf[:, :])

            last_act = [None]

            def act(*args, **kwargs):
                # force ACT program order so sqrt/exp table sets don't thrash
                ins = nc.scalar.activation(*args, **kwargs)
                if last_act[0] is not None:
                    add_dep_helper(ins.ins, last_act[0].ins, sync=False,
                                   reason="act stream order")
                last_act[0] = ins
                return ins

            # ---- blend + apply micro-tasks for one group of gg tiles ----
            def group_tasks(ub, t0, gg, col0):
                ns = 2 * gg
                state = {}

                def subtile(s):
                    if s == 0:
                        state["psb"] = psbp.tile([128, 272], dt.float32,
                                                 tag="psb", name="psbt")
                    psb = state["psb"]
                    for g in range(4):
                        # main + residual accumulate into the same psum block
                        ucol = (t0 + s // 2) * 1024 + 256 * g + 128 * (s % 2)
                        nc.tensor.matmul(
                            psb[:, 17 * s:17 * s + 17],
                            ub[:, ucol:ucol + 128],
                            tf_sb[:, 34 * g:34 * g + 17],
                            start=(g == 0), stop=False,
                        )
                        nc.tensor.matmul(
                            psb[:, 17 * s:17 * s + 17],
                            ub[:, ucol:ucol + 128],
                            tf_sb[:, 34 * g + 17:34 * g + 34],
                            start=False, stop=(g == 3),
                        )

                # apply is split into three tasks so each drain slot adds at
                # most a sub-us DVE burst between consecutive psum clamps
                def apply_a():
                    pv = state["psb"][:].rearrange("p (s j) -> p s j", j=17)
                    xr = app.tile([128, 48], dt.float32, tag="xr", name="xrt")
                    state["xr"] = xr
                    nc.gpsimd.dma_start(
                        out=xr[:, 0:3 * ns].rearrange("p (s c) -> p s c", c=3),
                        in_=xyz3[col0:col0 + 256 * gg, :].rearrange(
                            "(s p) c -> p s c", p=128),
                    )
                    rij = pv[:, 0:ns, 0:12].rearrange("p s (i j) -> p s i j", j=4)
                    R = rij[:, :, :, 0:3]
                    Xb = (xr[:, 0:3 * ns].rearrange("p (s c) -> p s c", c=3)
                          .broadcast_to((128, ns, 3, 3))
                          .rearrange("p s j i -> p s i j"))
                    t1 = app.tile([128, 144], dt.float32, tag="t1", name="t1t")
                    state["t1"] = t1
                    t1v = t1[:, 0:9 * ns].rearrange("p (s i j) -> p s i j", i=3, j=3)
                    nc.vector.tensor_mul(t1v, R, Xb)
                    rz = app.tile([128, 16], dt.float32, tag="rz", name="rzt")
                    state["rz"] = rz
                    nc.vector.reciprocal_approx_fast(out=rz[:, 0:ns],
                                                     in_=pv[:, 0:ns, 16])

                def apply_b():
                    pv = state["psb"][:].rearrange("p (s j) -> p s j", j=17)
                    rij = pv[:, 0:ns, 0:12].rearrange("p s (i j) -> p s i j", j=4)
                    Tr = rij[:, :, :, 3]
                    t1v = state["t1"][:, 0:9 * ns].rearrange(
                        "p (s i j) -> p s i j", i=3, j=3)
                    t2 = app.tile([128, 48], dt.float32, tag="t2", name="t2t")
                    state["t2"] = t2
                    t2v = t2[:, 0:3 * ns].rearrange("p (s i) -> p s i", i=3)
                    nc.vector.reduce_sum(t2v, t1v, axis=mybir.AxisListType.X)
                    nc.vector.tensor_add(t2v, t2v, Tr)

                def apply_c():
                    t2v = state["t2"][:, 0:3 * ns].rearrange("p (s i) -> p s i", i=3)
                    zb = (state["rz"][:, 0:ns].rearrange("p (s o) -> p s o", o=1)
                          .broadcast_to((128, ns, 3)))
                    nc.vector.tensor_mul(t2v, t2v, zb)
                    nc.gpsimd.dma_start(
                        out=out3[col0:col0 + 256 * gg, :].rearrange(
                            "(s p) c -> p s c", p=128),
                        in_=t2v,
                    )

                for s in range(ns):
                    yield lambda s=s: subtile(s)
                yield apply_a
                yield apply_b
                yield apply_c

            pending = deque()  # micro-tasks ready for PE/DVE

            def drain(n):
                k = 0
                while pending and k < n:
                    pending.popleft()()
                    k += 1

            tt = 0
            for ci, G in enumerate(chunks):
                ub = ubp.tile([128, 1024 * g_max], dt.bfloat16, tag="ub")
                groups = _chunks(G, GRP)
                xq = None
                for t in range(G):
                    col0 = (tt + t) * PTS_TILE
                    if t % GRP == 0:
                        gg = groups[t // GRP]
                        xq = xqp.tile([128, 2048], dt.bfloat16, tag="xq")
                        nc.sync.dma_start(
                            out=xq[0:KD, 0:gg * PTS_TILE],
                            in_=xyzq[:, col0:col0 + gg * PTS_TILE],
                        )
                    xoff = (t % GRP) * PTS_TILE
                    psd = psdp.tile([128, 1024], dt.float32, tag="psd")
                    for g in range(4):
                        nc.tensor.matmul(
                            psd[:, 256 * g:256 * (g + 1)],
                            bq_sb[0:KD, 128 * g:128 * g + 128],
                            xq[0:KD, xoff:xoff + PTS_TILE],
                            start=True, stop=True,
                        )
                    # s = sqrt(-2*p + eps)  (psum -> sbuf bf16); coincident
                    # point/bone pairs give sqrt(neg) = NaN, sanitized below
                    act(ub[:, t * 1024:(t + 1) * 1024], psd[:, :],
                        af.Sqrt, bias=eps_sb[:], scale=-2.0)
                    drain(TASKS_PER_SLOT)
                # u = exp(-sigma * s), in place, split per group; each part
                # releases that group's blend tasks so PE works during exp
                t0 = 0
                for gi, gg in enumerate(groups):
                    # DVE max(NaN, 0) = 0 (non-propagating): turns sqrt-NaN from
                    # fp-cancellation at coincident point/bone pairs into s=0
                    nc.vector.tensor_scalar_max(ub[:, t0 * 1024:(t0 + gg) * 1024],
                                                ub[:, t0 * 1024:(t0 + gg) * 1024],
                                                0.0)
                    act(ub[:, t0 * 1024:(t0 + gg) * 1024],
                        ub[:, t0 * 1024:(t0 + gg) * 1024],
                        af.Exp, bias=0.0, scale=-SIGMA)
                    pending.extend(group_tasks(ub, t0, gg, (tt + t0) * PTS_TILE))
                    drain(TASKS_PER_SLOT)
                    t0 += gg
                if ci == len(chunks) - 1:
                    drain(len(pending))
                tt += G
    nc.compile()
    _NC_CACHE[key] = nc
    return nc


def _cont2rotmat_np(rotcont):
    x = rotcont.reshape(-1, 3, 2).astype(np.float32)
    a1, a2 = x[..., 0], x[..., 1]
    b1 = a1 / (np.linalg.norm(a1, axis=-1, keepdims=True) + np.float32(1e-12))
    a2p = a2 - np.sum(b1 * a2, axis=-1, keepdims=True) * b1
    b2 = a2p / (np.linalg.norm(a2p, axis=-1, keepdims=True) + np.float32(1e-12))
    b3 = np.cross(b1, b2)
    return np.stack([b1, b2, b3], axis=-1).astype(np.float32)  # [K,3,3] cols


def _split_bf16(a):
    """a (fp32) -> (hi, lo) bf16 with hi + lo ~= a."""
    import ml_dtypes
    hi = a.astype(ml_dtypes.bfloat16)
    lo = (a - hi.astype(np.float32)).astype(ml_dtypes.bfloat16)
    return hi, lo


def host_prep(xyz_c, bone_locs, bone_transf, tidx, npc):
    """Build per-core input maps for the SPMD kernel."""
    import ml_dtypes
    bf16 = ml_dtypes.bfloat16
    xyz_c = np.ascontiguousarray(np.asarray(xyz_c, np.float32))
    bone_locs = np.asarray(bone_locs, np.float32)
    bone_transf = np.asarray(bone_transf, np.float32)
    ti = int(np.asarray(tidx))
    n = xyz_c.shape[0]
    npad = npc * N_CORES
    xyz_p = np.empty((npad, 3), np.float32)
    xyz_p[:n] = xyz_c
    xyz_p[n:] = xyz_c[0]

    params = bone_transf[ti]  # [512, 9]
    rot = _cont2rotmat_np(params[:, :6])  # [512,3,3]
    transl = params[:, 6:9]
    m17 = np.zeros((NB, 17), np.float32)
    m17[:, :12] = np.concatenate([rot, transl[:, :, None]], axis=-1).reshape(NB, 12)
    m17[:, 12:16] = np.array([0, 0, 0, 1], np.float32)
    m17[:, 16] = 1.0
    # split precision for the blend matmul: cols [0:17]=bf16 main,
    # [17:34]=bf16 residual per bone chunk.
    tf_h = np.zeros((128, 136), bf16)
    for g in range(4):
        blk = m17[128 * g:128 * (g + 1), :17]
        main, resid = _split_bf16(blk)
        tf_h[:, 34 * g:34 * g + 17] = main
        tf_h[:, 34 * g + 17:34 * g + 34] = resid

    # dist matmul operands, split precision over K=13 rows:
    #   rhs rows:  [xh(3), xl(3), xh(3), qh, ql, 1]     (q = -0.5|x|^2)
    #   lhsT rows: [bh(3), bh(3), bl(3), 1,  1,  bbh+?]
    # pairing: bh*xh + bh*xl + bl*xh + 1*qh + 1*ql + (bbh, bbl)*1
    bq_h = np.zeros((KD, 512), bf16)
    bh, blo = _split_bf16(bone_locs.T)  # [3,512]
    bbh, bbl = _split_bf16(-0.5 * np.sum(bone_locs * bone_locs, axis=1))
    bq_h[0:3] = bh
    bq_h[3:6] = bh
    bq_h[6:9] = blo
    bq_h[9:12] = blo
    bq_h[12] = 1.0
    bq_h[13] = 1.0
    bq_h[14] = bbh
    bq_h[15] = bbl

    in_maps = []
    for c in range(N_CORES):
        sl = xyz_p[c * npc:(c + 1) * npc]  # [npc,3]
        xh, xl = _split_bf16(sl.T)  # [3,npc]
        qh, ql = _split_bf16(-0.5 * np.sum(sl * sl, axis=1))
        x13 = np.empty((KD, npc), bf16)
        x13[0:3] = xh
        x13[3:6] = xl
        x13[6:9] = xh
        x13[9:12] = xl
        x13[12] = qh
        x13[13] = ql
        x13[14] = 1.0
        x13[15] = 1.0
        in_maps.append({
            "xyzq13": x13,
            "xyz3": sl.copy(),
            "bonesq": bq_h,
            "transf34": tf_h,
        })
    return in_maps


def kernel(xyz_c, bone_locs, bone_transf, tidx):
    xyz_c = np.asarray(xyz_c)
    n = xyz_c.shape[0]
    npc = ((n + N_CORES * PTS_TILE - 1) // (N_CORES * PTS_TILE)) * PTS_TILE
    nc = build_nc(npc)
    in_maps = host_prep(xyz_c, bone_locs, bone_transf, tidx, npc)
    res = run_bass_kernel_spmd(nc, in_maps, list(range(N_CORES)))
    out = np.concatenate([res.results[c]["out3"] for c in range(N_CORES)], axis=0)
    return np.ascontiguousarray(out[:n]).astype(np.float32)
